# revision 1
# baseline (speedup 1.0000x reference)
"""MoE layer (E=8 experts, top-2 routing) on 8 Trainium2 NeuronCores.

Strategy (expert-parallel, per the sharding hint):
  - The gate (T x D @ D x E, softmax, top-2, renorm) is computed on the host
    in fp32; it is ~0.01% of the FLOPs.
  - Tokens are dispatched by expert id ("all-to-all" done host-side): core e
    receives the tokens routed to expert e (padded to a common capacity C),
    together with expert e's weights in bf16.
  - Each core runs a Bass/Tile kernel computing
        y = combine_weight * (gelu(x @ w1 + b1) @ w2 + b2)
    with bf16 matmuls (fp32 PSUM accumulation) on the PE array:
      * phase H: H^T tiles (feature-major) = w1-chunk^T.T @ x^T-chunk,
        so no on-device transposes are needed (w1 natural layout is lhsT).
      * phase Y: token-major Y = H^T-chunk.T @ w2-chunk, which makes the
        per-token combine weight a per-partition scalar.
  - Host "unshard" is two gathers + an add (each token has exactly 2 slots).
"""

import sys
import types

import numpy as np
import ml_dtypes

import concourse.bass as bass
import concourse.mybir as mybir
from concourse import bacc
from concourse.tile import TileContext
from concourse.bass_utils import run_bass_kernel_spmd


def _ensure_antenv_hooks():
    """bass_utils imports antenv.axon_hooks when BASS_TRACE is set; this image
    may lack it. Provide the registry (with the real ctypes NTFF hook when
    available) so tracing works instead of crashing."""
    try:
        import antenv.axon_hooks  # noqa: F401
        return
    except ImportError:
        pass
    if "antenv" not in sys.modules:
        try:
            import antenv  # noqa: F401
        except ImportError:
            sys.modules["antenv"] = types.ModuleType("antenv")
    hooks = types.ModuleType("antenv.axon_hooks")
    state = {"hook": None}
    hooks.set_axon_ntff_profile_hook = lambda h: state.__setitem__("hook", h)
    hooks.get_axon_ntff_profile_hook = lambda: state["hook"]
    sys.modules["antenv"].axon_hooks = hooks
    sys.modules["antenv.axon_hooks"] = hooks
    try:
        from trn_agent_boot.trn_boot import _ntff_profile_via_ctypes
        hook = _ntff_profile_via_ctypes("/opt/axon/libaxon_pjrt.so")
        if hook is not None:
            hooks.set_axon_ntff_profile_hook(hook)
    except Exception:
        pass


_ensure_antenv_hooks()

P = 128
D = 1024
F = 4096
E = 8
TOPK = 2
NBLK = 512

_BF16 = ml_dtypes.bfloat16

_nc_cache: dict = {}
LAST = None  # BassKernelResults of the most recent run (for test harness)


def _build_moe_core(C: int) -> bass.Bass:
    """One-core SPMD program: FFN for C tokens with resident bf16 weights."""
    dt = mybir.dt
    nc = bacc.Bacc("TRN2", target_bir_lowering=False, debug=False)
    KO = D // P    # 8 contraction chunks for x @ w1
    FO = F // P    # 32 contraction chunks for h @ w2
    DN = D // NBLK  # 2 output-column blocks of w2
    GELU = mybir.ActivationFunctionType.Gelu

    xt = nc.dram_tensor("xt", [D, C], dt.bfloat16, kind="ExternalInput")
    # w1 host-pretiled per-fo: w1t[fo, p, ko, j] = w1[ko*P+p, fo*P+j], so each
    # 256KB fo-tile is one contiguous-per-partition DMA and the PE can start
    # after the first tile instead of the full 8MB.
    w1t = nc.dram_tensor("w1t", [FO, P, KO, P], dt.bfloat16,
                         kind="ExternalInput")
    w2 = nc.dram_tensor("w2", [F, D], dt.bfloat16, kind="ExternalInput")
    # b1/sc pre-packed partition-major on host so each DMA is one contiguous
    # descriptor per partition (the rearranged 1-D loads were 4B-strided).
    b1p = nc.dram_tensor("b1p", [P, FO], dt.float32, kind="ExternalInput")
    b2r = nc.dram_tensor("b2r", [P, D], dt.float32, kind="ExternalInput")
    scp = nc.dram_tensor("scp", [P, C // P], dt.float32, kind="ExternalInput")
    y = nc.dram_tensor("y", [C, D], dt.float32, kind="ExternalOutput")

    # Uniform 512-token blocks: smaller N makes the per-matmul LDWEIGHTS
    # (~97ns, FWL off in this toolchain) stop hiding inside the matmul
    # streaming window, measured +94ns/matmul at N=256.
    blocks = []
    off = 0
    while off < C:
        size = min(NBLK, C - off)
        blocks.append((off, size))
        off += size

    xt_r = xt.rearrange("(ko p) c -> p ko c", p=P)

    with TileContext(nc) as tc:
        with (
            tc.tile_pool(name="w", bufs=1) as wpool,
            tc.tile_pool(name="xin", bufs=2) as xpool,
            tc.tile_pool(name="h", bufs=1) as hpool,
            tc.tile_pool(name="yout", bufs=2) as ypool,
            tc.tile_pool(name="ph", bufs=3, space="PSUM") as phpool,
            tc.tile_pool(name="py", bufs=4, space="PSUM") as pypool,
            tc.tile_pool(name="pw", bufs=1, space="PSUM") as pwpool,
        ):
            # DMA issue order is the startup critical path: x block 0 (one
            # fused 1MB transfer) and the first w1 fo-tile gate the first
            # matmul; b1 is needed by the first gelu shortly after; the rest
            # (remaining w1, b2, sc, w2) only gate later work.
            KH = KO // 2  # x blocks load as two half-tiles (finer DMA deps)

            def load_x_block(n_off, n_size):
                xa = xpool.tile([P, KH, NBLK], dt.bfloat16, tag="xa")
                nc.sync.dma_start(
                    xa[:, :, :n_size], xt_r[:, :KH, n_off:n_off + n_size]
                )
                xb = xpool.tile([P, KH, NBLK], dt.bfloat16, tag="xb")
                nc.sync.dma_start(
                    xb[:, :, :n_size], xt_r[:, KH:, n_off:n_off + n_size]
                )
                return xa, xb

            def x_chunk(xts, ko):
                return xts[0][:, ko, :] if ko < KH else xts[1][:, ko - KH, :]

            # Warm the PE's HAM clock gate during the startup DMA window
            # with dummy matmuls on zeroed SBUF. Sized to the (deterministic)
            # PE-start -> data-ready delta of ~8us: an idle gap > ~3.4us
            # before the real stream would re-throttle the clock and cost
            # ~3us of cold matmuls.
            warm = wpool.tile([P, NBLK], dt.bfloat16, tag="warm")
            nc.gpsimd.memset(warm[:], 0.0)
            pwarm = pwpool.tile([P, NBLK], dt.float32, tag="pw")
            NWARM = 24
            for i in range(NWARM):
                nc.tensor.matmul(
                    pwarm[:], warm[:, :P], warm[:],
                    start=(i == 0), stop=(i == NWARM - 1),
                )

            xts0 = load_x_block(*blocks[0])

            w1sb = []
            for fo in range(FO):
                t_ = wpool.tile([P, KO, P], dt.bfloat16, tag=f"w1_{fo}")
                nc.sync.dma_start(t_[:], w1t[fo])
                w1sb.append(t_)
                if fo == 0:
                    b1sb = wpool.tile([P, FO], dt.float32, tag="b1")
                    nc.sync.dma_start(b1sb[:], b1p[:])

            b2sb = wpool.tile([P, D], dt.float32, tag="b2")
            nc.sync.dma_start(b2sb[:], b2r[:])
            scsb = wpool.tile([P, C // P], dt.float32, tag="sc")
            nc.sync.dma_start(scsb[:], scp[:])

            # w2 is only needed once the first Y phase starts (~60us in), so a
            # single consolidated tile/DMA is fine and keeps the live
            # semaphore count (and the exit drain's split-wait storm) small.
            w2sb = wpool.tile([P, FO, D], dt.bfloat16, tag="w2")
            nc.sync.dma_start(w2sb[:], w2.rearrange("(fo p) d -> p fo d", p=P))

            for bi, (n_off, n_size) in enumerate(blocks):
                xts = xts0 if bi == 0 else load_x_block(n_off, n_size)

                # H^T[f, t] = sum_d w1[d, f] * x^T[d, t], then gelu(+b1).
                htile = hpool.tile([P, FO, NBLK], dt.bfloat16, tag="h")
                for fo in range(FO):
                    ph = phpool.tile([P, NBLK], dt.float32, tag="ph")
                    for ko in range(KO):
                        nc.tensor.matmul(
                            ph[:, :n_size],
                            w1sb[fo][:, ko, :],
                            x_chunk(xts, ko)[:, :n_size],
                            start=(ko == 0),
                            stop=(ko == KO - 1),
                        )
                    nc.scalar.activation(
                        htile[:, fo, :n_size], ph[:, :n_size], GELU,
                        bias=b1sb[:, fo:fo + 1], scale=1.0,
                    )

                # Y[t, d] = sum_f H[t, f] * w2[f, d]; scale per token.
                for tb in range(n_size // P):
                    tbg = (n_off + tb * P) // P
                    ytile = ypool.tile([P, D], dt.float32, tag="y")
                    # dn-outer: the d-half 0 epilogue (bias add, scale, store)
                    # overlaps the d-half 1 matmuls, so only ~1.7us of
                    # epilogue trails the very last matmul of the kernel.
                    for dn in range(DN):
                        py = pypool.tile([P, NBLK], dt.float32, tag="py")
                        for fo in range(FO):
                            nc.tensor.matmul(
                                py[:],
                                htile[:, fo, tb * P:(tb + 1) * P],
                                w2sb[:, fo, dn * NBLK:(dn + 1) * NBLK],
                                start=(fo == 0),
                                stop=(fo == FO - 1),
                            )
                        dsl = slice(dn * NBLK, (dn + 1) * NBLK)
                        nc.vector.tensor_add(
                            ytile[:, dsl], py[:], b2sb[:, dsl]
                        )
                        nc.vector.tensor_scalar_mul(
                            ytile[:, dsl], ytile[:, dsl], scsb[:, tbg:tbg + 1]
                        )
                        nc.sync.dma_start(
                            y[n_off + tb * P:n_off + (tb + 1) * P, dsl],
                            ytile[:, dsl],
                        )
    nc.compile()
    return nc


def _route(flat, gate_w, gate_b):
    """fp32 gate matching the reference: softmax, top-2, renormalize."""
    logits = flat @ gate_w + gate_b
    m = logits.max(axis=1, keepdims=True)
    p = np.exp(logits - m, dtype=np.float32)
    probs = p / p.sum(axis=1, keepdims=True)
    ti = np.argsort(-probs, axis=1, kind="stable")[:, :TOPK]
    tp = np.take_along_axis(probs, ti, axis=1)
    sw = tp / (tp.sum(axis=1, keepdims=True) + np.float32(1e-9))
    return ti.astype(np.int64), sw.astype(np.float32)


def _dispatch(ti):
    """Slot assignment: (token, k) pair -> (expert, position-in-expert)."""
    Tn = ti.shape[0]
    flat_e = ti.ravel()
    order = np.argsort(flat_e, kind="stable")
    cnt = np.bincount(flat_e, minlength=E)
    starts = np.concatenate([[0], np.cumsum(cnt)[:-1]])
    ranks = np.arange(Tn * TOPK) - starts[flat_e[order]]
    pos = np.empty(Tn * TOPK, np.int64)
    pos[order] = ranks
    return flat_e, pos, cnt, starts, order


def _gelu_exact(v):
    try:
        from scipy.special import erf
        return 0.5 * v * (1.0 + erf(v / np.sqrt(2.0)))
    except ImportError:  # tanh approximation fallback (overflow tokens only)
        return 0.5 * v * (1.0 + np.tanh(
            0.7978845608028654 * (v + 0.044715 * v ** 3)))


def kernel(**inputs) -> np.ndarray:
    global LAST
    x = np.asarray(inputs["x"], np.float32)
    gate_w = np.asarray(inputs["gate_w"], np.float32)
    gate_b = np.asarray(inputs["gate_b"], np.float32)
    w1 = np.asarray(inputs["w1"], np.float32)
    b1 = np.asarray(inputs["b1"], np.float32)
    w2 = np.asarray(inputs["w2"], np.float32)
    b2 = np.asarray(inputs["b2"], np.float32)

    B, S, D_ = x.shape
    flat = x.reshape(-1, D_)
    Tn = flat.shape[0]

    ti, sw = _route(flat, gate_w, gate_b)
    flat_e, pos, cnt, starts, order = _dispatch(ti)

    # Capacity factor 1.0: each core processes exactly T*K/E token slots (the
    # SPMD program is uniform, so every core pays the max expert's cost —
    # capping at the mean keeps the device critical path balanced). The few
    # overflow tokens of the hottest experts are combined on the host in fp32.
    cap = (Tn * TOPK // E + P - 1) // P * P
    C = ((int(cnt.max()) + P - 1) // P) * P
    C = max(min(C, cap), P)

    xT_bf = np.ascontiguousarray(flat.T).astype(_BF16)  # [D, T]
    sw_flat = sw.ravel()

    in_maps = []
    overflow = []
    for e in range(E):
        pairs_all = order[starts[e]:starts[e] + cnt[e]]
        pairs = pairs_all[:C]
        if cnt[e] > C:
            overflow.append((e, pairs_all[C:]))
        n_e = len(pairs)
        toks = pairs // TOPK
        xt_e = np.zeros((D, C), _BF16)
        xt_e[:, :n_e] = xT_bf[:, toks]
        sc_e = np.zeros((C,), np.float32)
        sc_e[:n_e] = sw_flat[pairs]
        KO, FO = D // P, F // P
        w1_tiled = np.ascontiguousarray(
            w1[e].astype(_BF16).reshape(KO, P, FO, P).transpose(2, 1, 0, 3)
        )
        in_maps.append({
            "xt": xt_e,
            "w1t": w1_tiled,
            "w2": w2[e].astype(_BF16),
            "b1p": np.ascontiguousarray(b1[e].reshape(F // P, P).T),
            "b2r": np.ascontiguousarray(
                np.broadcast_to(b2[e], (P, D))
            ).astype(np.float32),
            "scp": np.ascontiguousarray(sc_e.reshape(C // P, P).T),
        })

    nc = _nc_cache.get(C)
    if nc is None:
        nc = _build_moe_core(C)
        _nc_cache[C] = nc

    LAST = run_bass_kernel_spmd(nc, in_maps, core_ids=list(range(E)))
    Yall = np.stack([np.asarray(LAST.results[i]["y"]) for i in range(E)])

    # Combine: device slots via two gathers; host fp32 FFN for overflow.
    in_cap = pos < C
    contrib = np.zeros((Tn * TOPK, D_), np.float32)
    idx = np.nonzero(in_cap)[0]
    contrib[idx] = Yall[flat_e[idx], pos[idx]]
    out = contrib[0::TOPK] + contrib[1::TOPK]
    for e, over in overflow:
        toks = over // TOPK
        h = _gelu_exact(flat[toks] @ w1[e] + b1[e])
        y_e = h @ w2[e] + b2[e]
        out[toks] += sw_flat[over][:, None] * y_e
    return out.reshape(B, S, D_).astype(np.float32)



# revision 9
# speedup vs baseline: 1.0479x; 1.0479x over previous
"""MoE layer (E=8 experts, top-2 routing) on 8 Trainium2 NeuronCores.

Strategy (expert-parallel, per the sharding hint):
  - fp32 gate on host (~0.01% of FLOPs); tokens dispatched by expert id
    host-side; core e gets expert e's tokens padded to capacity C=2048
    (the mean load), with per-expert slots SORTED BY COMBINE WEIGHT desc.
  - Device blocks of 512 tokens: the first 3 blocks (high combine weight)
    run bf16 matmuls; the LAST block (lowest weights) runs fp8-e4m3
    DoubleRow matmuls (2 MACs/cell/cycle = 2x PE throughput). The fp8
    quantization noise (~5% per slot) lands only on the ~8% of combine-
    weight mass carried by the bottom block, keeping global rel err
    ~1.5e-2 (< 2e-2 gate).
  - Weight residency: bf16 w1/w2 resident in SBUF; the fp8 copies are
    streamed per-tile during the fp8 block (8 MB over ~30 us, ~130 GB/s).
  - Overflow beyond capacity = the lowest-weight slots, combined on host
    in exact fp32.
Phase H computes feature-major H^T tiles (w1 natural layout is lhsT);
phase Y is token-major so the per-token combine weight is a per-partition
scalar. fp8 scales: w1*512, w2*1024, x*32 (powers of 2, descaled exactly
in the activation / epilogue constants).
"""

import sys
import types

import numpy as np
import ml_dtypes

import concourse.bass as bass
import concourse.mybir as mybir
from concourse import bacc
from concourse.tile import TileContext
from concourse.bass_utils import run_bass_kernel_spmd


def _ensure_antenv_hooks():
    """bass_utils imports antenv.axon_hooks when BASS_TRACE is set; this image
    may lack it. Provide the registry (with the real ctypes NTFF hook when
    available) so tracing works instead of crashing."""
    try:
        import antenv.axon_hooks  # noqa: F401
        return
    except ImportError:
        pass
    if "antenv" not in sys.modules:
        try:
            import antenv  # noqa: F401
        except ImportError:
            sys.modules["antenv"] = types.ModuleType("antenv")
    hooks = types.ModuleType("antenv.axon_hooks")
    state = {"hook": None}
    hooks.set_axon_ntff_profile_hook = lambda h: state.__setitem__("hook", h)
    hooks.get_axon_ntff_profile_hook = lambda: state["hook"]
    sys.modules["antenv"].axon_hooks = hooks
    sys.modules["antenv.axon_hooks"] = hooks
    try:
        from trn_agent_boot.trn_boot import _ntff_profile_via_ctypes
        hook = _ntff_profile_via_ctypes("/opt/axon/libaxon_pjrt.so")
        if hook is not None:
            hooks.set_axon_ntff_profile_hook(hook)
    except Exception:
        pass


_ensure_antenv_hooks()

P = 128
D = 1024
F = 4096
E = 8
TOPK = 2
NBLK = 512
NF8 = 1          # number of fp8 blocks (of C // NBLK total)
KDR = D // 256   # 4 DoubleRow contraction chunks for x @ w1
FOP = F // 256   # 16 DoubleRow contraction chunks for h @ w2
DN = D // NBLK   # 2 output-column blocks of w2
W1S = 512.0      # host scale on fp8 w1
W2S = 1024.0     # host scale on fp8 w2
XS = 32.0        # host scale on fp8 x
HDS = 1.0 / (W1S * XS)    # fp8 phase-H activation input descale

_BF16 = ml_dtypes.bfloat16
_E4 = ml_dtypes.float8_e4m3

_nc_cache: dict = {}
LAST = None  # BassKernelResults of the most recent run (for test harness)


def _build_moe_core(C: int, nf8: int = NF8) -> bass.Bass:
    """One-core SPMD program: FFN for C tokens, mixed bf16/fp8 blocks."""
    dt = mybir.dt
    DR = mybir.MatmulPerfMode.DoubleRow
    nc = bacc.Bacc("TRN2", target_bir_lowering=False, debug=False)
    KO = D // P    # 8 bf16 contraction chunks for x @ w1
    FO = F // P    # 32 bf16 contraction chunks for h @ w2
    DN = D // NBLK  # 2 output-column blocks of w2
    GELU = mybir.ActivationFunctionType.Gelu
    NB = C // NBLK

    xt = nc.dram_tensor("xt", [D, C], dt.bfloat16, kind="ExternalInput")
    # w1 host-pretiled per-fo: w1t[fo, p, ko, j] = w1[ko*P+p, fo*P+j], so each
    # 256KB fo-tile is one contiguous-per-partition DMA and the PE can start
    # after the first tile instead of the full 8MB.
    w1t = nc.dram_tensor("w1t", [FO, P, KO, P], dt.bfloat16,
                         kind="ExternalInput")
    w2 = nc.dram_tensor("w2", [F, D], dt.bfloat16, kind="ExternalInput")
    # b1/sc pre-packed partition-major on host so each DMA is one contiguous
    # descriptor per partition.
    b1p = nc.dram_tensor("b1p", [P, FO], dt.float32, kind="ExternalInput")
    b2r = nc.dram_tensor("b2r", [P, D], dt.float32, kind="ExternalInput")
    scp = nc.dram_tensor("scp", [P, C // P], dt.float32, kind="ExternalInput")
    if nf8 > 0:
        # fp8 copies for the low-weight blocks (streamed, not resident):
        # w1t8[fo, p, kdr, i, j] = 512*w1[kdr*256 + i*128 + p, fo*128 + j]
        w1t8 = nc.dram_tensor("w1t8", [FO, P, KDR, 2, P], dt.float8e4,
                              kind="ExternalInput")
        # w2t8[dn, j, p, i, d] = 1024*w2[(2j+i)*128 + p, dn*512 + d]
        w2t8 = nc.dram_tensor("w2t8", [DN, FOP, P, 2, NBLK], dt.float8e4,
                              kind="ExternalInput")
        # x8[p, kdr, i, t] = 32*x[fp8 slot t][kdr*256 + i*128 + p]
        x8d = nc.dram_tensor("x8d", [P, KDR, 2, nf8 * NBLK], dt.float8e4,
                             kind="ExternalInput")
        b2r8 = nc.dram_tensor("b2r8", [P, D], dt.float32,
                              kind="ExternalInput")
    y = nc.dram_tensor("y", [C, D], dt.float32, kind="ExternalOutput")

    blocks = []
    off = 0
    while off < C:
        size = min(NBLK, C - off)
        blocks.append((off, size))
        off += size
    classes = ["bf16"] * (NB - nf8) + ["fp8"] * nf8

    xt_r = xt.rearrange("(ko p) c -> p ko c", p=P)

    with TileContext(nc) as tc:
        with (
            tc.tile_pool(name="w", bufs=1) as wpool,
            tc.tile_pool(name="w8s", bufs=3) as w8pool,
            tc.tile_pool(name="w28s", bufs=4) as w28pool,
            tc.tile_pool(name="xin", bufs=2) as xpool,
            tc.tile_pool(name="h", bufs=1) as hpool,
            tc.tile_pool(name="yout", bufs=2) as ypool,
            tc.tile_pool(name="ph", bufs=3, space="PSUM") as phpool,
            tc.tile_pool(name="py", bufs=4, space="PSUM") as pypool,
            tc.tile_pool(name="pw", bufs=1, space="PSUM") as pwpool,
        ):
            KH = KO // 2  # x blocks load as two half-tiles (finer DMA deps)

            def load_x_block(n_off, n_size):
                xa = xpool.tile([P, KH, NBLK], dt.bfloat16, tag="xa")
                nc.sync.dma_start(
                    xa[:, :, :n_size], xt_r[:, :KH, n_off:n_off + n_size]
                )
                xb = xpool.tile([P, KH, NBLK], dt.bfloat16, tag="xb")
                nc.sync.dma_start(
                    xb[:, :, :n_size], xt_r[:, KH:, n_off:n_off + n_size]
                )
                return xa, xb

            def load_x8_block(fi):
                tsl = slice(fi * NBLK, (fi + 1) * NBLK)
                xa = xpool.tile([P, 2, 2, NBLK], dt.float8e4, tag="xa")
                nc.sync.dma_start(xa[:], x8d[:, :2, :, tsl])
                xb = xpool.tile([P, 2, 2, NBLK], dt.float8e4, tag="xb")
                nc.sync.dma_start(xb[:], x8d[:, 2:, :, tsl])
                return xa, xb

            def x_chunk(xts, ko):
                return xts[0][:, ko, :] if ko < KH else xts[1][:, ko - KH, :]

            def x8_chunk(xts, kdr):
                return xts[0][:, kdr] if kdr < 2 else xts[1][:, kdr - 2]

            # Warm the PE's HAM clock gate during the startup DMA window.
            warm = wpool.tile([P, NBLK], dt.bfloat16, tag="warm")
            nc.gpsimd.memset(warm[:], 0.0)
            pwarm = pwpool.tile([P, NBLK], dt.float32, tag="pw")
            NWARM = 24
            for i in range(NWARM):
                nc.tensor.matmul(
                    pwarm[:], warm[:, :P], warm[:],
                    start=(i == 0), stop=(i == NWARM - 1),
                )

            xts0 = load_x_block(*blocks[0])

            w1sb = []
            for fo in range(FO):
                t_ = wpool.tile([P, KO, P], dt.bfloat16, tag=f"w1_{fo}")
                nc.sync.dma_start(t_[:], w1t[fo])
                w1sb.append(t_)
                if fo == 0:
                    b1sb = wpool.tile([P, FO], dt.float32, tag="b1")
                    nc.sync.dma_start(b1sb[:], b1p[:])

            b2sb = wpool.tile([P, D], dt.float32, tag="b2")
            nc.sync.dma_start(b2sb[:], b2r[:])
            if nf8 > 0:
                b2sb8 = wpool.tile([P, D], dt.float32, tag="b28")
                nc.sync.dma_start(b2sb8[:], b2r8[:])
            scsb = wpool.tile([P, C // P], dt.float32, tag="sc")
            nc.sync.dma_start(scsb[:], scp[:])

            # w2 is only needed once the first Y phase starts (~60us in).
            w2sb = wpool.tile([P, FO, D], dt.bfloat16, tag="w2")
            nc.sync.dma_start(w2sb[:], w2.rearrange("(fo p) d -> p fo d", p=P))

            fi = 0  # fp8 block ordinal
            for bi, (n_off, n_size) in enumerate(blocks):
                if classes[bi] == "bf16":
                    xts = xts0 if bi == 0 else load_x_block(n_off, n_size)

                    # H^T[f, t] = sum_d w1[d, f] * x^T[d, t]; gelu(+b1).
                    htile = hpool.tile([P, FO, NBLK], dt.bfloat16, tag="h")
                    for fo in range(FO):
                        ph = phpool.tile([P, NBLK], dt.float32, tag="ph")
                        for ko in range(KO):
                            nc.tensor.matmul(
                                ph[:, :n_size],
                                w1sb[fo][:, ko, :],
                                x_chunk(xts, ko)[:, :n_size],
                                start=(ko == 0),
                                stop=(ko == KO - 1),
                            )
                        nc.scalar.activation(
                            htile[:, fo, :n_size], ph[:, :n_size], GELU,
                            bias=b1sb[:, fo:fo + 1], scale=1.0,
                        )

                    # Y[t, d] = sum_f H[t, f] * w2[f, d]; scale per token.
                    for tb in range(n_size // P):
                        tbg = (n_off + tb * P) // P
                        for dn in range(DN):
                            py = pypool.tile([P, NBLK], dt.float32, tag="py")
                            for fo in range(FO):
                                nc.tensor.matmul(
                                    py[:],
                                    htile[:, fo, tb * P:(tb + 1) * P],
                                    w2sb[:, fo, dn * NBLK:(dn + 1) * NBLK],
                                    start=(fo == 0),
                                    stop=(fo == FO - 1),
                                )
                            dsl = slice(dn * NBLK, (dn + 1) * NBLK)
                            ytile = ypool.tile([P, NBLK], dt.float32, tag="y")
                            nc.vector.tensor_add(ytile[:], py[:], b2sb[:, dsl])
                            nc.vector.tensor_scalar_mul(
                                ytile[:], ytile[:], scsb[:, tbg:tbg + 1]
                            )
                            nc.sync.dma_start(
                                y[n_off + tb * P:n_off + (tb + 1) * P, dsl],
                                ytile[:],
                            )
                else:
                    # fp8 DoubleRow block (lowest combine weights).
                    xts = load_x8_block(fi)

                    htile8 = hpool.tile([P, FO, NBLK], dt.float8e4, tag="h")
                    for fo in range(FO):
                        w18 = w8pool.tile([P, KDR, 2, P], dt.float8e4,
                                          tag="w18")
                        nc.sync.dma_start(w18[:], w1t8[fo])
                        ph = phpool.tile([P, NBLK], dt.float32, tag="ph")
                        for kdr in range(KDR):
                            nc.tensor.matmul(
                                ph[:],
                                w18[:, kdr],
                                x8_chunk(xts, kdr),
                                start=(kdr == 0),
                                stop=(kdr == KDR - 1),
                                perf_mode=DR,
                            )
                        nc.scalar.activation(
                            htile8[:, fo, :], ph[:], GELU,
                            bias=b1sb[:, fo:fo + 1], scale=HDS,
                        )

                    # Y phase: dn/j outer (w2 chunks streamed once), tb inner
                    # with 4 concurrently-open PSUM groups.
                    for dn in range(DN):
                        dsl = slice(dn * NBLK, (dn + 1) * NBLK)
                        pys = [pypool.tile([P, NBLK], dt.float32, tag="py",
                                           name=f"py8_{dn}_{tb}")
                               for tb in range(n_size // P)]
                        for j in range(FOP):
                            w28 = w28pool.tile([P, 2, NBLK], dt.float8e4,
                                               tag="w28")
                            nc.sync.dma_start(w28[:], w2t8[dn, j])
                            for tb in range(n_size // P):
                                nc.tensor.matmul(
                                    pys[tb][:],
                                    htile8[:, 2 * j:2 * j + 2,
                                           tb * P:(tb + 1) * P],
                                    w28[:],
                                    start=(j == 0),
                                    stop=(j == FOP - 1),
                                    perf_mode=DR,
                                )
                        for tb in range(n_size // P):
                            tbg = (n_off + tb * P) // P
                            ytile = ypool.tile([P, NBLK], dt.float32, tag="y")
                            nc.vector.tensor_add(
                                ytile[:], pys[tb][:], b2sb8[:, dsl]
                            )
                            nc.vector.tensor_scalar_mul(
                                ytile[:], ytile[:], scsb[:, tbg:tbg + 1]
                            )
                            nc.sync.dma_start(
                                y[n_off + tb * P:n_off + (tb + 1) * P, dsl],
                                ytile[:],
                            )
                    fi += 1
    nc.compile()
    return nc


def _route(flat, gate_w, gate_b):
    """fp32 gate matching the reference: softmax, top-2, renormalize."""
    logits = flat @ gate_w + gate_b
    m = logits.max(axis=1, keepdims=True)
    p = np.exp(logits - m, dtype=np.float32)
    probs = p / p.sum(axis=1, keepdims=True)
    ti = np.argsort(-probs, axis=1, kind="stable")[:, :TOPK]
    tp = np.take_along_axis(probs, ti, axis=1)
    sw = tp / (tp.sum(axis=1, keepdims=True) + np.float32(1e-9))
    return ti.astype(np.int64), sw.astype(np.float32)


def _dispatch(ti, sw):
    """Slot assignment: per expert, slots sorted by combine weight DESC so
    the trailing device block holds the lowest weights (fp8 class) and
    overflow beyond capacity (host-exact) is the tail."""
    Tn = ti.shape[0]
    flat_e = ti.ravel()
    flat_w = sw.ravel()
    cnt = np.bincount(flat_e, minlength=E)
    starts = np.concatenate([[0], np.cumsum(cnt)[:-1]])
    # sort by (expert asc, weight desc); stable for determinism
    order = np.lexsort((-flat_w, flat_e))
    pos = np.empty(Tn * TOPK, np.int64)
    pos[order] = np.arange(Tn * TOPK) - starts[flat_e[order]]
    return flat_e, pos, cnt, starts, order


def _gelu_exact(v):
    try:
        from scipy.special import erf
        return 0.5 * v * (1.0 + erf(v / np.sqrt(2.0)))
    except ImportError:  # tanh approximation fallback (overflow tokens only)
        return 0.5 * v * (1.0 + np.tanh(
            0.7978845608028654 * (v + 0.044715 * v ** 3)))


def kernel(**inputs) -> np.ndarray:
    global LAST
    x = np.asarray(inputs["x"], np.float32)
    gate_w = np.asarray(inputs["gate_w"], np.float32)
    gate_b = np.asarray(inputs["gate_b"], np.float32)
    w1 = np.asarray(inputs["w1"], np.float32)
    b1 = np.asarray(inputs["b1"], np.float32)
    w2 = np.asarray(inputs["w2"], np.float32)
    b2 = np.asarray(inputs["b2"], np.float32)

    B, S, D_ = x.shape
    flat = x.reshape(-1, D_)
    Tn = flat.shape[0]

    ti, sw = _route(flat, gate_w, gate_b)
    flat_e, pos, cnt, starts, order = _dispatch(ti, sw)

    # Capacity factor 1.0: each core processes exactly T*K/E token slots.
    cap = (Tn * TOPK // E + P - 1) // P * P
    C = ((int(cnt.max()) + P - 1) // P) * P
    C = max(min(C, cap), P)
    nf8 = NF8 if C == cap else 0  # fp8 path sized for the standard capacity

    xT_bf = np.ascontiguousarray(flat.T).astype(_BF16)  # [D, T]
    xT8 = np.ascontiguousarray(
        np.clip(flat.T * XS, -240.0, 240.0)).astype(_E4)
    sw_flat = sw.ravel()
    f8_start = C - nf8 * NBLK

    in_maps = []
    overflow = []
    for e in range(E):
        pairs_all = order[starts[e]:starts[e] + cnt[e]]
        pairs = pairs_all[:C]
        if cnt[e] > C:
            overflow.append((e, pairs_all[C:]))
        n_e = len(pairs)
        toks = pairs // TOPK
        xt_e = np.zeros((D, C), _BF16)
        xt_e[:, :n_e] = xT_bf[:, toks]
        sc_e = np.zeros((C,), np.float32)
        sc_e[:n_e] = sw_flat[pairs]
        KO, FO = D // P, F // P
        w1_tiled = np.ascontiguousarray(
            w1[e].astype(_BF16).reshape(KO, P, FO, P).transpose(2, 1, 0, 3)
        )
        im = {
            "xt": xt_e,
            "w1t": w1_tiled,
            "w2": w2[e].astype(_BF16),
            "b1p": np.ascontiguousarray(b1[e].reshape(F // P, P).T),
            "b2r": np.ascontiguousarray(
                np.broadcast_to(b2[e], (P, D))
            ).astype(np.float32),
        }
        if nf8 > 0:
            # fp8 block inputs: slots [f8_start:C), x scaled by XS
            x8_e = np.zeros((D, nf8 * NBLK), _E4)
            n8 = max(0, n_e - f8_start)
            if n8 > 0:
                x8_e[:, :n8] = xT8[:, toks[f8_start:f8_start + n8]]
            sc_e[f8_start:] *= np.float32(1.0 / W2S)
            # w1t8[fo,p,kdr,i,j] = W1S*w1[kdr*256 + i*128 + p, fo*128 + j]
            im["w1t8"] = np.ascontiguousarray(
                np.clip(w1[e] * W1S, -240, 240).astype(_E4)
                .reshape(KDR, 2, P, FO, P).transpose(3, 2, 0, 1, 4)
            )
            # w2t8[dn, j, p, i, d] = W2S*w2[(2j+i)*128 + p, dn*512 + d]
            im["w2t8"] = np.ascontiguousarray(
                np.clip(w2[e] * W2S, -240, 240).astype(_E4)
                .reshape(FOP, 2, P, DN, NBLK).transpose(3, 0, 2, 1, 4)
            )
            # x8_e [D, n8cols] -> [p, kdr, i, t]
            im["x8d"] = np.ascontiguousarray(
                x8_e.reshape(KDR, 2, P, nf8 * NBLK).transpose(2, 0, 1, 3)
            )
            im["b2r8"] = np.ascontiguousarray(
                np.broadcast_to(b2[e] * W2S, (P, D))
            ).astype(np.float32)
        im["scp"] = np.ascontiguousarray(sc_e.reshape(C // P, P).T)
        in_maps.append(im)

    nc = _nc_cache.get((C, nf8))
    if nc is None:
        nc = _build_moe_core(C, nf8)
        _nc_cache[(C, nf8)] = nc

    LAST = run_bass_kernel_spmd(nc, in_maps, core_ids=list(range(E)))
    Yall = np.stack([np.asarray(LAST.results[i]["y"]) for i in range(E)])

    # Combine: device slots via two gathers; host fp32 FFN for overflow.
    in_cap = pos < C
    contrib = np.zeros((Tn * TOPK, D_), np.float32)
    idx = np.nonzero(in_cap)[0]
    contrib[idx] = Yall[flat_e[idx], pos[idx]]
    out = contrib[0::TOPK] + contrib[1::TOPK]
    for e, over in overflow:
        toks = over // TOPK
        h = _gelu_exact(flat[toks] @ w1[e] + b1[e])
        y_e = h @ w2[e] + b2[e]
        out[toks] += sw_flat[over][:, None] * y_e
    return out.reshape(B, S, D_).astype(np.float32)


# revision 10
# speedup vs baseline: 1.0827x; 1.0332x over previous
"""MoE layer (E=8 experts, top-2 routing) on 8 Trainium2 NeuronCores.

Strategy (expert-parallel, per the sharding hint):
  - fp32 gate on host (~0.01% of FLOPs); tokens dispatched by expert id
    host-side; core e gets expert e's tokens padded to capacity C=2048
    (the mean load), with per-expert slots SORTED BY COMBINE WEIGHT desc.
  - Device blocks of 512 tokens: the first 3 blocks (high combine weight)
    run bf16 matmuls; the LAST block (lowest weights) runs fp8-e4m3
    DoubleRow matmuls (2 MACs/cell/cycle = 2x PE throughput). The fp8
    quantization noise (~5% per slot) lands only on the ~8% of combine-
    weight mass carried by the bottom block, keeping global rel err
    ~1.5e-2 (< 2e-2 gate).
  - Weight residency: bf16 w1/w2 resident in SBUF; the fp8 copies are
    streamed per-tile during the fp8 block (8 MB over ~30 us, ~130 GB/s).
  - Overflow beyond capacity = the lowest-weight slots, combined on host
    in exact fp32.
Phase H computes feature-major H^T tiles (w1 natural layout is lhsT);
phase Y is token-major so the per-token combine weight is a per-partition
scalar. fp8 scales: w1*512, w2*1024, x*32 (powers of 2, descaled exactly
in the activation / epilogue constants).
"""

import sys
import types

import numpy as np
import ml_dtypes

import concourse.bass as bass
import concourse.mybir as mybir
from concourse import bacc
from concourse.tile import TileContext
from concourse.bass_utils import run_bass_kernel_spmd


def _ensure_antenv_hooks():
    """bass_utils imports antenv.axon_hooks when BASS_TRACE is set; this image
    may lack it. Provide the registry (with the real ctypes NTFF hook when
    available) so tracing works instead of crashing."""
    try:
        import antenv.axon_hooks  # noqa: F401
        return
    except ImportError:
        pass
    if "antenv" not in sys.modules:
        try:
            import antenv  # noqa: F401
        except ImportError:
            sys.modules["antenv"] = types.ModuleType("antenv")
    hooks = types.ModuleType("antenv.axon_hooks")
    state = {"hook": None}
    hooks.set_axon_ntff_profile_hook = lambda h: state.__setitem__("hook", h)
    hooks.get_axon_ntff_profile_hook = lambda: state["hook"]
    sys.modules["antenv"].axon_hooks = hooks
    sys.modules["antenv.axon_hooks"] = hooks
    try:
        from trn_agent_boot.trn_boot import _ntff_profile_via_ctypes
        hook = _ntff_profile_via_ctypes("/opt/axon/libaxon_pjrt.so")
        if hook is not None:
            hooks.set_axon_ntff_profile_hook(hook)
    except Exception:
        pass


_ensure_antenv_hooks()

P = 128
D = 1024
F = 4096
E = 8
TOPK = 2
NBLK = 512
NF8 = 1          # number of fp8 blocks (of C // NBLK total)
KDR = D // 256   # 4 DoubleRow contraction chunks for x @ w1
FOP = F // 256   # 16 DoubleRow contraction chunks for h @ w2
DN = D // NBLK   # 2 output-column blocks of w2
W1S = 512.0      # host scale on fp8 w1
W2S = 1024.0     # host scale on fp8 w2
XS = 32.0        # host scale on fp8 x
HDS = 1.0 / (W1S * XS)    # fp8 phase-H activation input descale

_BF16 = ml_dtypes.bfloat16
_E4 = ml_dtypes.float8_e4m3

_nc_cache: dict = {}
LAST = None  # BassKernelResults of the most recent run (for test harness)


def _build_moe_core(C: int, nf8: int = NF8) -> bass.Bass:
    """One-core SPMD program: FFN for C tokens, mixed bf16/fp8 blocks."""
    dt = mybir.dt
    DR = mybir.MatmulPerfMode.DoubleRow
    nc = bacc.Bacc("TRN2", target_bir_lowering=False, debug=False)
    KO = D // P    # 8 bf16 contraction chunks for x @ w1
    FO = F // P    # 32 bf16 contraction chunks for h @ w2
    DN = D // NBLK  # 2 output-column blocks of w2
    GELU = mybir.ActivationFunctionType.Gelu
    NB = C // NBLK

    xt = nc.dram_tensor("xt", [D, C], dt.bfloat16, kind="ExternalInput")
    # w1 host-pretiled per-fo: w1t[fo, p, ko, j] = w1[ko*P+p, fo*P+j], so each
    # 256KB fo-tile is one contiguous-per-partition DMA and the PE can start
    # after the first tile instead of the full 8MB.
    w1t = nc.dram_tensor("w1t", [FO, P, KO, P], dt.bfloat16,
                         kind="ExternalInput")
    w2 = nc.dram_tensor("w2", [F, D], dt.bfloat16, kind="ExternalInput")
    # b1/sc pre-packed partition-major on host so each DMA is one contiguous
    # descriptor per partition.
    b1p = nc.dram_tensor("b1p", [P, FO], dt.float32, kind="ExternalInput")
    b2r = nc.dram_tensor("b2r", [P, D], dt.float32, kind="ExternalInput")
    scp = nc.dram_tensor("scp", [P, C // P], dt.float32, kind="ExternalInput")
    if nf8 > 0:
        # fp8 copies for the low-weight blocks (streamed, not resident):
        # w1t8[fo, p, kdr, i, j] = 512*w1[kdr*256 + i*128 + p, fo*128 + j]
        w1t8 = nc.dram_tensor("w1t8", [FO, P, KDR, 2, P], dt.float8e4,
                              kind="ExternalInput")
        # w2t8[dn, j, p, i, d] = 1024*w2[(2j+i)*128 + p, dn*512 + d]
        w2t8 = nc.dram_tensor("w2t8", [DN, FOP, P, 2, NBLK], dt.float8e4,
                              kind="ExternalInput")
        # x8[p, kdr, i, t] = 32*x[fp8 slot t][kdr*256 + i*128 + p]
        x8d = nc.dram_tensor("x8d", [P, KDR, 2, nf8 * NBLK], dt.float8e4,
                             kind="ExternalInput")
        b2r8 = nc.dram_tensor("b2r8", [P, D], dt.float32,
                              kind="ExternalInput")
    y = nc.dram_tensor("y", [C, D], dt.float32, kind="ExternalOutput")

    blocks = []
    off = 0
    while off < C:
        size = min(NBLK, C - off)
        blocks.append((off, size))
        off += size
    classes = ["bf16"] * (NB - nf8) + ["fp8"] * nf8

    xt_r = xt.rearrange("(ko p) c -> p ko c", p=P)

    with TileContext(nc) as tc:
        with (
            tc.tile_pool(name="w", bufs=1) as wpool,
            tc.tile_pool(name="w8s", bufs=8) as w8pool,
            tc.tile_pool(name="w28s", bufs=8) as w28pool,
            tc.tile_pool(name="xin", bufs=2) as xpool,
            tc.tile_pool(name="h", bufs=1) as hpool,
            tc.tile_pool(name="yout", bufs=2) as ypool,
            tc.tile_pool(name="ph", bufs=3, space="PSUM") as phpool,
            tc.tile_pool(name="py", bufs=4, space="PSUM") as pypool,
            tc.tile_pool(name="pw", bufs=1, space="PSUM") as pwpool,
        ):
            KH = KO // 2  # x blocks load as two half-tiles (finer DMA deps)

            def load_x_block(n_off, n_size):
                xa = xpool.tile([P, KH, NBLK], dt.bfloat16, tag="xa")
                nc.sync.dma_start(
                    xa[:, :, :n_size], xt_r[:, :KH, n_off:n_off + n_size]
                )
                xb = xpool.tile([P, KH, NBLK], dt.bfloat16, tag="xb")
                nc.sync.dma_start(
                    xb[:, :, :n_size], xt_r[:, KH:, n_off:n_off + n_size]
                )
                return xa, xb

            def load_x8_block(fi):
                tsl = slice(fi * NBLK, (fi + 1) * NBLK)
                xa = xpool.tile([P, 2, 2, NBLK], dt.float8e4, tag="xa")
                nc.sync.dma_start(xa[:], x8d[:, :2, :, tsl])
                xb = xpool.tile([P, 2, 2, NBLK], dt.float8e4, tag="xb")
                nc.sync.dma_start(xb[:], x8d[:, 2:, :, tsl])
                return xa, xb

            def x_chunk(xts, ko):
                return xts[0][:, ko, :] if ko < KH else xts[1][:, ko - KH, :]

            def x8_chunk(xts, kdr):
                return xts[0][:, kdr] if kdr < 2 else xts[1][:, kdr - 2]

            # Warm the PE's HAM clock gate during the startup DMA window.
            warm = wpool.tile([P, NBLK], dt.bfloat16, tag="warm")
            nc.gpsimd.memset(warm[:], 0.0)
            pwarm = pwpool.tile([P, NBLK], dt.float32, tag="pw")
            NWARM = 24
            for i in range(NWARM):
                nc.tensor.matmul(
                    pwarm[:], warm[:, :P], warm[:],
                    start=(i == 0), stop=(i == NWARM - 1),
                )

            xts0 = load_x_block(*blocks[0])

            w1sb = []
            for fo in range(FO):
                t_ = wpool.tile([P, KO, P], dt.bfloat16, tag=f"w1_{fo}")
                nc.sync.dma_start(t_[:], w1t[fo])
                w1sb.append(t_)
                if fo == 0:
                    b1sb = wpool.tile([P, FO], dt.float32, tag="b1")
                    nc.sync.dma_start(b1sb[:], b1p[:])

            b2sb = wpool.tile([P, D], dt.float32, tag="b2")
            nc.sync.dma_start(b2sb[:], b2r[:])
            if nf8 > 0:
                b2sb8 = wpool.tile([P, D], dt.float32, tag="b28")
                nc.sync.dma_start(b2sb8[:], b2r8[:])
            scsb = wpool.tile([P, C // P], dt.float32, tag="sc")
            nc.sync.dma_start(scsb[:], scp[:])

            # w2 is only needed once the first Y phase starts (~60us in).
            w2sb = wpool.tile([P, FO, D], dt.bfloat16, tag="w2")
            nc.sync.dma_start(w2sb[:], w2.rearrange("(fo p) d -> p fo d", p=P))

            fi = 0  # fp8 block ordinal
            for bi, (n_off, n_size) in enumerate(blocks):
                if classes[bi] == "bf16":
                    xts = xts0 if bi == 0 else load_x_block(n_off, n_size)

                    # H^T[f, t] = sum_d w1[d, f] * x^T[d, t]; gelu(+b1).
                    htile = hpool.tile([P, FO, NBLK], dt.bfloat16, tag="h")
                    for fo in range(FO):
                        ph = phpool.tile([P, NBLK], dt.float32, tag="ph")
                        for ko in range(KO):
                            nc.tensor.matmul(
                                ph[:, :n_size],
                                w1sb[fo][:, ko, :],
                                x_chunk(xts, ko)[:, :n_size],
                                start=(ko == 0),
                                stop=(ko == KO - 1),
                            )
                        nc.scalar.activation(
                            htile[:, fo, :n_size], ph[:, :n_size], GELU,
                            bias=b1sb[:, fo:fo + 1], scale=1.0,
                        )

                    # Y[t, d] = sum_f H[t, f] * w2[f, d]; scale per token.
                    for tb in range(n_size // P):
                        tbg = (n_off + tb * P) // P
                        for dn in range(DN):
                            py = pypool.tile([P, NBLK], dt.float32, tag="py")
                            for fo in range(FO):
                                nc.tensor.matmul(
                                    py[:],
                                    htile[:, fo, tb * P:(tb + 1) * P],
                                    w2sb[:, fo, dn * NBLK:(dn + 1) * NBLK],
                                    start=(fo == 0),
                                    stop=(fo == FO - 1),
                                )
                            dsl = slice(dn * NBLK, (dn + 1) * NBLK)
                            ytile = ypool.tile([P, NBLK], dt.float32, tag="y")
                            nc.vector.tensor_add(ytile[:], py[:], b2sb[:, dsl])
                            nc.vector.tensor_scalar_mul(
                                ytile[:], ytile[:], scsb[:, tbg:tbg + 1]
                            )
                            nc.sync.dma_start(
                                y[n_off + tb * P:n_off + (tb + 1) * P, dsl],
                                ytile[:],
                            )
                else:
                    # fp8 DoubleRow block (lowest combine weights).
                    xts = load_x8_block(fi)

                    htile8 = hpool.tile([P, FO, NBLK], dt.float8e4, tag="h")
                    for fo in range(FO):
                        w18 = w8pool.tile([P, KDR, 2, P], dt.float8e4,
                                          tag="w18")
                        nc.sync.dma_start(w18[:], w1t8[fo])
                        ph = phpool.tile([P, NBLK], dt.float32, tag="ph")
                        for kdr in range(KDR):
                            nc.tensor.matmul(
                                ph[:],
                                w18[:, kdr],
                                x8_chunk(xts, kdr),
                                start=(kdr == 0),
                                stop=(kdr == KDR - 1),
                                perf_mode=DR,
                            )
                        nc.scalar.activation(
                            htile8[:, fo, :], ph[:], GELU,
                            bias=b1sb[:, fo:fo + 1], scale=HDS,
                        )

                    # Y phase: dn/j outer (w2 chunks streamed once), tb inner
                    # with 4 concurrently-open PSUM groups.
                    for dn in range(DN):
                        dsl = slice(dn * NBLK, (dn + 1) * NBLK)
                        pys = [pypool.tile([P, NBLK], dt.float32, tag="py",
                                           name=f"py8_{dn}_{tb}")
                               for tb in range(n_size // P)]
                        for j in range(FOP):
                            w28 = w28pool.tile([P, 2, NBLK], dt.float8e4,
                                               tag="w28")
                            nc.sync.dma_start(w28[:], w2t8[dn, j])
                            for tb in range(n_size // P):
                                nc.tensor.matmul(
                                    pys[tb][:],
                                    htile8[:, 2 * j:2 * j + 2,
                                           tb * P:(tb + 1) * P],
                                    w28[:],
                                    start=(j == 0),
                                    stop=(j == FOP - 1),
                                    perf_mode=DR,
                                )
                        for tb in range(n_size // P):
                            tbg = (n_off + tb * P) // P
                            ytile = ypool.tile([P, NBLK], dt.float32, tag="y")
                            nc.vector.tensor_add(
                                ytile[:], pys[tb][:], b2sb8[:, dsl]
                            )
                            nc.vector.tensor_scalar_mul(
                                ytile[:], ytile[:], scsb[:, tbg:tbg + 1]
                            )
                            nc.sync.dma_start(
                                y[n_off + tb * P:n_off + (tb + 1) * P, dsl],
                                ytile[:],
                            )
                    fi += 1
    nc.compile()
    return nc


def _route(flat, gate_w, gate_b):
    """fp32 gate matching the reference: softmax, top-2, renormalize."""
    logits = flat @ gate_w + gate_b
    m = logits.max(axis=1, keepdims=True)
    p = np.exp(logits - m, dtype=np.float32)
    probs = p / p.sum(axis=1, keepdims=True)
    ti = np.argsort(-probs, axis=1, kind="stable")[:, :TOPK]
    tp = np.take_along_axis(probs, ti, axis=1)
    sw = tp / (tp.sum(axis=1, keepdims=True) + np.float32(1e-9))
    return ti.astype(np.int64), sw.astype(np.float32)


def _dispatch(ti, sw):
    """Slot assignment: per expert, slots sorted by combine weight DESC so
    the trailing device block holds the lowest weights (fp8 class) and
    overflow beyond capacity (host-exact) is the tail."""
    Tn = ti.shape[0]
    flat_e = ti.ravel()
    flat_w = sw.ravel()
    cnt = np.bincount(flat_e, minlength=E)
    starts = np.concatenate([[0], np.cumsum(cnt)[:-1]])
    # sort by (expert asc, weight desc); stable for determinism
    order = np.lexsort((-flat_w, flat_e))
    pos = np.empty(Tn * TOPK, np.int64)
    pos[order] = np.arange(Tn * TOPK) - starts[flat_e[order]]
    return flat_e, pos, cnt, starts, order


def _gelu_exact(v):
    try:
        from scipy.special import erf
        return 0.5 * v * (1.0 + erf(v / np.sqrt(2.0)))
    except ImportError:  # tanh approximation fallback (overflow tokens only)
        return 0.5 * v * (1.0 + np.tanh(
            0.7978845608028654 * (v + 0.044715 * v ** 3)))


def kernel(**inputs) -> np.ndarray:
    global LAST
    x = np.asarray(inputs["x"], np.float32)
    gate_w = np.asarray(inputs["gate_w"], np.float32)
    gate_b = np.asarray(inputs["gate_b"], np.float32)
    w1 = np.asarray(inputs["w1"], np.float32)
    b1 = np.asarray(inputs["b1"], np.float32)
    w2 = np.asarray(inputs["w2"], np.float32)
    b2 = np.asarray(inputs["b2"], np.float32)

    B, S, D_ = x.shape
    flat = x.reshape(-1, D_)
    Tn = flat.shape[0]

    ti, sw = _route(flat, gate_w, gate_b)
    flat_e, pos, cnt, starts, order = _dispatch(ti, sw)

    # Capacity factor 1.0: each core processes exactly T*K/E token slots.
    cap = (Tn * TOPK // E + P - 1) // P * P
    C = ((int(cnt.max()) + P - 1) // P) * P
    C = max(min(C, cap), P)
    nf8 = NF8 if C == cap else 0  # fp8 path sized for the standard capacity

    xT_bf = np.ascontiguousarray(flat.T).astype(_BF16)  # [D, T]
    xT8 = np.ascontiguousarray(
        np.clip(flat.T * XS, -240.0, 240.0)).astype(_E4)
    sw_flat = sw.ravel()
    f8_start = C - nf8 * NBLK

    in_maps = []
    overflow = []
    for e in range(E):
        pairs_all = order[starts[e]:starts[e] + cnt[e]]
        pairs = pairs_all[:C]
        if cnt[e] > C:
            overflow.append((e, pairs_all[C:]))
        n_e = len(pairs)
        toks = pairs // TOPK
        xt_e = np.zeros((D, C), _BF16)
        xt_e[:, :n_e] = xT_bf[:, toks]
        sc_e = np.zeros((C,), np.float32)
        sc_e[:n_e] = sw_flat[pairs]
        KO, FO = D // P, F // P
        w1_tiled = np.ascontiguousarray(
            w1[e].astype(_BF16).reshape(KO, P, FO, P).transpose(2, 1, 0, 3)
        )
        im = {
            "xt": xt_e,
            "w1t": w1_tiled,
            "w2": w2[e].astype(_BF16),
            "b1p": np.ascontiguousarray(b1[e].reshape(F // P, P).T),
            "b2r": np.ascontiguousarray(
                np.broadcast_to(b2[e], (P, D))
            ).astype(np.float32),
        }
        if nf8 > 0:
            # fp8 block inputs: slots [f8_start:C), x scaled by XS
            x8_e = np.zeros((D, nf8 * NBLK), _E4)
            n8 = max(0, n_e - f8_start)
            if n8 > 0:
                x8_e[:, :n8] = xT8[:, toks[f8_start:f8_start + n8]]
            sc_e[f8_start:] *= np.float32(1.0 / W2S)
            # w1t8[fo,p,kdr,i,j] = W1S*w1[kdr*256 + i*128 + p, fo*128 + j]
            im["w1t8"] = np.ascontiguousarray(
                np.clip(w1[e] * W1S, -240, 240).astype(_E4)
                .reshape(KDR, 2, P, FO, P).transpose(3, 2, 0, 1, 4)
            )
            # w2t8[dn, j, p, i, d] = W2S*w2[(2j+i)*128 + p, dn*512 + d]
            im["w2t8"] = np.ascontiguousarray(
                np.clip(w2[e] * W2S, -240, 240).astype(_E4)
                .reshape(FOP, 2, P, DN, NBLK).transpose(3, 0, 2, 1, 4)
            )
            # x8_e [D, n8cols] -> [p, kdr, i, t]
            im["x8d"] = np.ascontiguousarray(
                x8_e.reshape(KDR, 2, P, nf8 * NBLK).transpose(2, 0, 1, 3)
            )
            im["b2r8"] = np.ascontiguousarray(
                np.broadcast_to(b2[e] * W2S, (P, D))
            ).astype(np.float32)
        im["scp"] = np.ascontiguousarray(sc_e.reshape(C // P, P).T)
        in_maps.append(im)

    nc = _nc_cache.get((C, nf8))
    if nc is None:
        nc = _build_moe_core(C, nf8)
        _nc_cache[(C, nf8)] = nc

    LAST = run_bass_kernel_spmd(nc, in_maps, core_ids=list(range(E)))
    Yall = np.stack([np.asarray(LAST.results[i]["y"]) for i in range(E)])

    # Combine: device slots via two gathers; host fp32 FFN for overflow.
    in_cap = pos < C
    contrib = np.zeros((Tn * TOPK, D_), np.float32)
    idx = np.nonzero(in_cap)[0]
    contrib[idx] = Yall[flat_e[idx], pos[idx]]
    out = contrib[0::TOPK] + contrib[1::TOPK]
    for e, over in overflow:
        toks = over // TOPK
        h = _gelu_exact(flat[toks] @ w1[e] + b1[e])
        y_e = h @ w2[e] + b2[e]
        out[toks] += sw_flat[over][:, None] * y_e
    return out.reshape(B, S, D_).astype(np.float32)
